# revision 1
# baseline (speedup 1.0000x reference)
"""Trainium2 Bass kernel for DynamicImpactEncoder.

impact[b,t,c] = alpha[c] * sum_{s>=t} events[b,s,c] * exp(-(s-t)/decay[c])
Computed as a backward linear recurrence y[t] = alpha*x[t] + r*y[t+1]
(r = exp(-1/decay)) using the DVE tensor_tensor_scan instruction with
reversed, channel-strided access patterns over [batch, (t,c)] tiles.

Sharding: pure data parallel over batch (512 -> 8 cores x 64).
"""

import numpy as np

B, T, C = 512, 16384, 3
N_CORES = 8
B_SHARD = B // N_CORES          # 64
ROW = T * C                     # 49152 floats per batch row
CHUNK_T = 2048                  # timesteps per tile
CHUNK_X = CHUNK_T * C           # 6144 floats per tile row
N_CHUNKS = T // CHUNK_T

_CACHE = {}


def _build(r_vals, alpha_vals):
    from concourse import bacc
    import concourse.tile as tile
    import concourse.mybir as mybir

    nc = bacc.Bacc(trn_type="TRN2", target_bir_lowering=False,
                   num_devices=N_CORES)
    x = nc.declare_dram_parameter("x", [B_SHARD, ROW], mybir.dt.float32,
                                  isOutput=False)
    y = nc.declare_dram_parameter("y", [B_SHARD, ROW], mybir.dt.float32,
                                  isOutput=True)

    with tile.TileContext(nc) as tc:
        with tc.tile_pool(name="cst", bufs=1) as cpool, \
             tc.tile_pool(name="io", bufs=3) as pool:
            rts = []
            for c in range(C):
                rt = cpool.tile([B_SHARD, 1], mybir.dt.float32, name=f"r{c}")
                nc.vector.memset(rt[:], float(r_vals[c]))
                rts.append(rt)

            prev_yt = None
            for k in range(N_CHUNKS - 1, -1, -1):
                sl = slice(k * CHUNK_X, (k + 1) * CHUNK_X)
                xt = pool.tile([B_SHARD, CHUNK_X], mybir.dt.float32, name="xt")
                yt = pool.tile([B_SHARD, CHUNK_X], mybir.dt.float32, name="yt")
                nc.sync.dma_start(xt[:], x.ap()[:, sl])
                for c in range(C):
                    nc.scalar.mul(xt[:, c::3], xt[:, c::3],
                                  float(alpha_vals[c]))
                for c in range(C):
                    init = 0.0 if prev_yt is None else prev_yt[:, c:c + 1]
                    nc.vector.tensor_tensor_scan(
                        yt[:, c::3][:, ::-1],
                        rts[c][:].to_broadcast([B_SHARD, CHUNK_T]),
                        xt[:, c::3][:, ::-1],
                        init,
                        mybir.AluOpType.mult,
                        mybir.AluOpType.add,
                    )
                nc.sync.dma_start(y.ap()[:, sl], yt[:])
                prev_yt = yt

    nc.compile()
    return nc


def kernel(events, time_decay, alpha):
    import jax.numpy as jnp
    from concourse.bass_utils import run_bass_kernel_spmd

    r_vals = np.asarray(jnp.exp(-1.0 / jnp.asarray(time_decay,
                                                   dtype=jnp.float32)))
    alpha_vals = np.asarray(alpha, dtype=np.float32)

    key = (tuple(r_vals.tolist()), tuple(alpha_vals.tolist()))
    if key not in _CACHE:
        _CACHE[key] = _build(r_vals, alpha_vals)
    nc = _CACHE[key]

    ev = np.ascontiguousarray(events, dtype=np.float32).reshape(B, ROW)
    in_maps = [{"x": ev[i * B_SHARD:(i + 1) * B_SHARD]}
               for i in range(N_CORES)]
    res = run_bass_kernel_spmd(nc, in_maps, list(range(N_CORES)))
    out = np.concatenate([res.results[i]["y"] for i in range(N_CORES)],
                         axis=0)
    return out.reshape(B, T, C)


# revision 2
# speedup vs baseline: 24.5069x; 24.5069x over previous
"""Trainium2 Bass kernel for DynamicImpactEncoder.

impact[b,t,c] = alpha[c] * sum_{s>=t} events[b,s,c] * exp(-(s-t)/decay[c])
Computed as a backward linear recurrence y[t] = alpha*x[t] + r*y[t+1]
(r = exp(-1/decay)) using the DVE tensor_tensor_scan instruction with
reversed, channel-strided access patterns over [batch, (t,c)] tiles.

Sharding: pure data parallel over batch (512 -> 8 cores x 64).
"""

import numpy as np

B, T, C = 512, 16384, 3
N_CORES = 8
B_SHARD = B // N_CORES          # 64
ROW = T * C                     # 49152 floats per batch row
CHUNK_T = 2048                  # timesteps per tile
CHUNK_X = CHUNK_T * C           # 6144 floats per tile row
N_CHUNKS = T // CHUNK_T

_CACHE = {}


def _build(r_vals, alpha_vals, repeat=1):
    from concourse import bacc
    import concourse.tile as tile
    import concourse.mybir as mybir

    nc = bacc.Bacc(trn_type="TRN2", target_bir_lowering=False,
                   num_devices=N_CORES)
    x = nc.declare_dram_parameter("x", [B_SHARD, ROW], mybir.dt.float32,
                                  isOutput=False)
    y = nc.declare_dram_parameter("y", [B_SHARD, ROW], mybir.dt.float32,
                                  isOutput=True)

    with tile.TileContext(nc) as tc:
        with tc.tile_pool(name="cst", bufs=1) as cpool, \
             tc.tile_pool(name="io", bufs=3) as pool:
            rts = []
            for c in range(C):
                rt = cpool.tile([B_SHARD, 1], mybir.dt.float32, name=f"r{c}")
                nc.vector.memset(rt[:], float(r_vals[c]))
                rts.append(rt)

            for rep in range(repeat):
                prev_yt = None
                for k in range(N_CHUNKS - 1, -1, -1):
                    sl = slice(k * CHUNK_X, (k + 1) * CHUNK_X)
                    xt = pool.tile([B_SHARD, CHUNK_X], mybir.dt.float32,
                                   name="xt")
                    yt = pool.tile([B_SHARD, CHUNK_X], mybir.dt.float32,
                                   name="yt")
                    nc.sync.dma_start(xt[:], x.ap()[:, sl])
                    for c in range(C):
                        nc.scalar.mul(xt[:, c::3], xt[:, c::3],
                                      float(alpha_vals[c]))
                    for c in range(C):
                        init = 0.0 if prev_yt is None else prev_yt[:, c:c + 1]
                        nc.vector.tensor_tensor_scan(
                            yt[:, c::3][:, ::-1],
                            rts[c][:].to_broadcast([B_SHARD, CHUNK_T]),
                            xt[:, c::3][:, ::-1],
                            init,
                            mybir.AluOpType.mult,
                            mybir.AluOpType.add,
                        )
                    nc.sync.dma_start(y.ap()[:, sl], yt[:])
                    prev_yt = yt

    nc.compile()
    return nc


def kernel(events, time_decay, alpha):
    import jax.numpy as jnp
    from concourse.bass_utils import run_bass_kernel_spmd

    r_vals = np.asarray(jnp.exp(-1.0 / jnp.asarray(time_decay,
                                                   dtype=jnp.float32)))
    alpha_vals = np.asarray(alpha, dtype=np.float32)

    key = (tuple(r_vals.tolist()), tuple(alpha_vals.tolist()))
    if key not in _CACHE:
        _CACHE[key] = _build(r_vals, alpha_vals)
    nc = _CACHE[key]

    ev = np.ascontiguousarray(events, dtype=np.float32).reshape(B, ROW)
    in_maps = [{"x": ev[i * B_SHARD:(i + 1) * B_SHARD]}
               for i in range(N_CORES)]
    res = run_bass_kernel_spmd(nc, in_maps, list(range(N_CORES)))
    out = np.concatenate([res.results[i]["y"] for i in range(N_CORES)],
                         axis=0)
    return out.reshape(B, T, C)
